# revision 1
# baseline (speedup 1.0000x reference)
"""Causal multi-head attention (B=2, S=2048, D=1024, H=16, Dh=64) on 8 trn2
NeuronCores.

Sharding: tensor-parallel over (batch x head-group). Core c handles batch
c//4 and heads [4*(c%4), 4*(c%4)+4). Each core computes its heads' Q/K/V
projections, causal softmax attention, and a partial output projection
(row-parallel Wo). Host sums the 4 partials per batch and adds bo.

Device-side layout ("scores-transposed"): the contraction dim always sits on
partitions so no transposes are ever needed:
  qT/kT: [head-dim on partitions, seq free]   (from W.T @ x.T)
  v:     [seq on partitions, head-dim free]   (from x @ Wv)
  scoresT[k, q] = kT-block.T @ qT-block       (k seq on partitions)
  softmax: exp on ACT (no max subtraction - scores are O(3) here); the row
           sums ride along the v matmul via an appended ones column; the
           1/sum broadcast is a K=1 matmul; normalization is one vector mul.
  out    = h_norm-blocks.T @ Wo-rows          (partial, summed on host)

All matmul operands are float32r (full PE rate at N>=256, ~1e-4 rel err).
Constraint honored throughout: matmul PSUM destinations must start at
partition 0, and a matmul with start=True zeroes its whole 2KB bank, so
even/odd head accumulation groups live in separate banks.
"""

import numpy as np

import concourse.bacc as bacc
import concourse.mybir as mybir
import concourse.tile as tile
from concourse import bass2jax

F32 = mybir.dt.float32
F32R = mybir.dt.float32r

B, S, D = 2, 2048, 1024
H_PER_CORE = 4          # heads per core
DH = 64                 # head dim
FW = H_PER_CORE * DH    # 256: per-core projection width
N_CORES = 8
QCHUNK = 512            # q columns processed per chunk
NQC = S // QCHUNK       # 4 chunks
KT = S // 128           # 16 k-tiles
# v_t per-s-tile layout, per head pair p at offset p*193:
#   [0:64]=v_even  [64:65]=1 (even sums row 64)  [65:66]=1 (odd sums row 0)
#   [66:129]=unused  [129:193]=v_odd (odd out rows 64:128)
VSEG = 193
VBLK = 2 * VSEG         # 386 per s-tile


def build_nc(reps: int = 1, loop_trips: int = 1):
    nc = bacc.Bacc("TRN2", target_bir_lowering=False, debug=False)

    xT = nc.dram_tensor("xT", [D, S], F32R, kind="ExternalInput")
    wq = nc.dram_tensor("wq", [D, FW], F32R, kind="ExternalInput")
    wk = nc.dram_tensor("wk", [D, FW], F32R, kind="ExternalInput")
    wv = nc.dram_tensor("wv", [D, FW], F32R, kind="ExternalInput")
    wo = nc.dram_tensor("wo", [FW, D], F32R, kind="ExternalInput")
    bq = nc.dram_tensor("bq", [FW, 1], F32, kind="ExternalInput")
    bk = nc.dram_tensor("bk", [FW, 1], F32, kind="ExternalInput")
    bvb = nc.dram_tensor("bvb", [128, FW], F32, kind="ExternalInput")
    ones = nc.dram_tensor("ones", [128, 128], F32R, kind="ExternalInput")
    maskg = nc.dram_tensor("maskg", [128, 2048], F32R, kind="ExternalInput")
    out = nc.dram_tensor("out", [S, D], F32, kind="ExternalOutput")

    with tile.TileContext(nc) as tc, nc.allow_low_precision(
            reason="float32r matmul operands carry reduced mantissas by design"):
        if loop_trips > 1:
            with tc.For_i(0, loop_trips, 1):
                _emit_body(nc, tc, xT, wq, wk, wv, wo, bq, bk, bvb, ones,
                           maskg, out)
        else:
            for _ in range(reps):
                _emit_body(nc, tc, xT, wq, wk, wv, wo, bq, bk, bvb, ones,
                           maskg, out)
    nc.compile()
    return nc


def _emit_body(nc, tc, xT, wq, wk, wv, wo, bq, bk, bvb, ones, maskg, out):
    """One full attention pass.

    Projections are emitted chunk-by-chunk and *interleaved into the previous
    chunk's softmax waves* (causality: attention chunk J only needs k/v/q
    chunks 0..J). Each engine executes its instructions in emission order, so
    the interleave is what keeps PE busy while ACT evaluates exp, and keeps
    the PE array HAM-warm. Output projection of chunk J is likewise deferred
    into chunk J+1's waves.
    """
    with tc.tile_pool(name="wpool", bufs=1) as wpool, \
         tc.tile_pool(name="qkv", bufs=1) as qkv, \
         tc.tile_pool(name="xtp", bufs=3) as xtp, \
         tc.tile_pool(name="spp", bufs=2, space="PSUM") as spp, \
         tc.tile_pool(name="hpp", bufs=1, space="PSUM") as hpp, \
         tc.tile_pool(name="sh512", bufs=2, space="PSUM") as sh512, \
         tc.tile_pool(name="expw", bufs=5) as expw_pool, \
         tc.tile_pool(name="sm", bufs=2) as sm_pool, \
         tc.tile_pool(name="hn", bufs=4) as hn_pool, \
         tc.tile_pool(name="op", bufs=4) as op_pool:
        # --- weight/aux tiles; DMAs ordered by first use ---
        wq_t = wpool.tile([128, 8 * FW], F32R)   # [d-in-tile, (d-tile, f)]
        wk_t = wpool.tile([128, 8 * FW], F32R)
        wv_t = wpool.tile([128, 8 * FW], F32R)
        wo_t = wpool.tile([128, 2 * D], F32R)    # [fw-in-tile, (fw-tile, n)]
        bq_t = wpool.tile([128, 2], F32)
        bk_t = wpool.tile([128, 2], F32)
        bvb_t = wpool.tile([128, FW], F32)
        ones_t = wpool.tile([128, 128], F32R)
        maskg_t = wpool.tile([128, 2048], F32R)
        qT = [qkv.tile([128, S], F32R, name=f"qT{p}") for p in range(2)]
        kT = [qkv.tile([128, S], F32R, name=f"kT{p}") for p in range(2)]
        v_t = qkv.tile([128, KT * VBLK], F32R)

        def dma_xt(J):
            x_t = xtp.tile([128, 8 * 512], F32R, name="xt")
            nc.sync.dma_start(
                out=x_t[:].rearrange("p (t s) -> p t s", t=8),
                in_=xT[:, J * 512:(J + 1) * 512].rearrange(
                    "(t p) s -> p t s", p=128))
            return [x_t[:, d * 512:(d + 1) * 512] for d in range(8)]

        # DMAs ordered by first use; one descriptor per tensor.
        nc.sync.dma_start(out=wk_t[:].rearrange("p (t f) -> p t f", t=8),
                          in_=wk[:].rearrange("(t p) f -> p t f", p=128))
        xt_cur = dma_xt(0)
        nc.sync.dma_start(out=wq_t[:].rearrange("p (t f) -> p t f", t=8),
                          in_=wq[:].rearrange("(t p) f -> p t f", p=128))
        nc.sync.dma_start(out=bk_t[:].rearrange("p (t f) -> p t f", t=2),
                          in_=bk[:].rearrange("(t p) f -> p t f", p=128))
        nc.sync.dma_start(out=bq_t[:].rearrange("p (t f) -> p t f", t=2),
                          in_=bq[:].rearrange("(t p) f -> p t f", p=128))
        nc.sync.dma_start(out=ones_t[:], in_=ones[:])
        nc.sync.dma_start(out=wv_t[:].rearrange("p (t f) -> p t f", t=8),
                          in_=wv[:].rearrange("(t p) f -> p t f", p=128))
        nc.sync.dma_start(out=bvb_t[:], in_=bvb[:])
        nc.sync.dma_start(out=maskg_t[:], in_=maskg[:])
        nc.sync.dma_start(out=wo_t[:].rearrange("p (t f) -> p t f", t=2),
                          in_=wo[:].rearrange("(t p) f -> p t f", p=128))
        # ones columns of v_t (positions 64,65 within each 193-block)
        nc.vector.tensor_copy(
            v_t[:].rearrange("x (s p b) -> x s p b", s=KT, p=2)[:, :, :, 64:66],
            ones_t[:, 0:64].rearrange("x (s p b) -> x s p b", s=KT, p=2))

        def proj_tasks(J, xt):
            """12 closures: one PE accumulation group + vector epilogue each."""
            tasks = []
            scs = slice(J * 512, (J + 1) * 512)
            for dst, w_t, b_t in ((kT, wk_t, bk_t), (qT, wq_t, bq_t)):
                for p in range(2):
                    def qk_group(dst=dst, w_t=w_t, b_t=b_t, p=p):
                        pt = sh512.tile([128, 512], F32, name="pt",
                                        tag="sh512")
                        for d in range(8):
                            nc.tensor.matmul(
                                pt[:],
                                w_t[:, d * FW + p * 128:
                                    d * FW + (p + 1) * 128],
                                xt[d][:],
                                start=(d == 0), stop=(d == 7),
                            )
                        nc.vector.tensor_scalar_add(
                            dst[p][:, scs], pt[:], b_t[:, p:p + 1])
                    tasks.append(qk_group)
            for j in range(4):
                def v_group(j=j):
                    st = 4 * J + j
                    pt = sh512.tile([128, FW], F32, name="pt", tag="sh512")
                    for d in range(8):
                        nc.tensor.matmul(
                            pt[:],
                            xt[d][:, j * 128:(j + 1) * 128],
                            wv_t[:, d * FW:(d + 1) * FW],
                            start=(d == 0), stop=(d == 7),
                        )
                    seg = v_t[:, st * VBLK:(st + 1) * VBLK].rearrange(
                        "x (p b) -> x p b", p=2)
                    pt4 = pt[:].rearrange("x (h c) -> x h c", h=4)
                    bv4 = bvb_t[:].rearrange("x (h c) -> x h c", h=4)
                    nc.vector.tensor_add(seg[:, :, 0:64], pt4[:, 0:4:2, :],
                                         bv4[:, 0:4:2, :])
                    nc.vector.tensor_add(seg[:, :, 129:193],
                                         pt4[:, 1:4:2, :], bv4[:, 1:4:2, :])
                tasks.append(v_group)
            return tasks

        def outproj_tasks(J, hn_t):
            """8 closures: 2-matmul group + copy; 4 DMA-out closures."""
            tasks = []
            o_tiles = {}
            for m in range(4):
                o_t = op_pool.tile([128, D], F32, name="o_t")
                o_tiles[m] = o_t
                for n in range(2):
                    def o_group(m=m, n=n, o_t=o_t):
                        o_ps = sh512.tile([128, 512], F32, name="o_ps",
                                          tag="sh512")
                        for p in range(2):
                            nc.tensor.matmul(
                                o_ps[:],
                                hn_t[p][:, m * 128:(m + 1) * 128],
                                wo_t[:, p * D + n * 512:
                                     p * D + (n + 1) * 512],
                                start=(p == 0), stop=(p == 1),
                            )
                        cp = nc.vector.tensor_copy if n == 0 else \
                            nc.scalar.copy
                        cp(o_t[:, n * 512:(n + 1) * 512], o_ps[:])
                        if n == 1:
                            nc.sync.dma_start(
                                out=out[J * 512 + m * 128:
                                        J * 512 + (m + 1) * 128, :],
                                in_=o_t[:])
                    tasks.append(o_group)
            return tasks

        filler = []
        for t in proj_tasks(0, xt_cur):
            t()  # chunk 0 projections run up front
        for J in range(NQC):
            n_ki = 4 * J + 4
            qs = slice(J * 512, (J + 1) * 512)
            if J + 1 < NQC:
                xt_next = dma_xt(J + 1)
                filler.extend(proj_tasks(J + 1, xt_next))
            n_waves = 2 * n_ki
            wave_no = 0
            fill_total = len(filler)
            fill_done = 0
            hn_t = [None, None]
            for p in range(2):
                # h_ps bank 0: even head rows [0:64]=h, [64:65]=sums
                # h_ps bank 1: odd head  rows [0:1]=sums, [64:128]=h
                h_ps = hpp.tile([128, 1024], F32, name="h_ps")
                vbase = p * VSEG
                def emit_wv(ki, ew, off):
                    # h + sums in one matmul per head (ones col in v_t)
                    nc.tensor.matmul(
                        h_ps[0:65, off:512],
                        v_t[:, ki * VBLK + vbase: ki * VBLK + vbase + 65],
                        ew[:, off:512],
                        start=(ki == 0), stop=(ki == n_ki - 1),
                    )
                    nc.tensor.matmul(
                        h_ps[0:128, 512 + off:1024],
                        v_t[:, ki * VBLK + vbase + 65:
                            ki * VBLK + vbase + VSEG],
                        ew[:, 512 + off:1024],
                        start=(ki == 0), stop=(ki == n_ki - 1),
                    )

                # Software pipeline (lag 2): the wv matmuls of wave w are
                # emitted after the scores of wave w+2, so the PE stream
                # never waits on a freshly-issued exp (engines are FIFO).
                # Diagonal tiles (m >= 0) only touch columns [off:512],
                # off = 128*m: everything below is causally dead.
                pending = []
                for ki in range(n_ki):
                    m = ki - 4 * J
                    off = 128 * m if m > 0 else 0
                    sc_ps = spp.tile([128, 1024], F32, name="sc_ps")
                    # scores^T: row-tiled head pair (K=64 each)
                    nc.tensor.matmul(
                        sc_ps[:, off:512],
                        kT[p][0:64, ki * 128:(ki + 1) * 128],
                        qT[p][0:64, J * 512 + off:(J + 1) * 512],
                        start=True, stop=True, tile_position=(0, 0),
                    )
                    nc.tensor.matmul(
                        sc_ps[:, 512 + off:1024],
                        kT[p][64:128, ki * 128:(ki + 1) * 128],
                        qT[p][64:128, J * 512 + off:(J + 1) * 512],
                        start=True, stop=True, tile_position=(64, 0),
                    )
                    ew = expw_pool.tile([128, 1024], F32R, name="ew")
                    if off == 0:
                        nc.scalar.activation(
                            ew[:], sc_ps[:], mybir.ActivationFunctionType.Exp)
                    else:
                        nc.scalar.activation(
                            ew[:, off:512], sc_ps[:, off:512],
                            mybir.ActivationFunctionType.Exp)
                        nc.scalar.activation(
                            ew[:, 512 + off:1024], sc_ps[:, 512 + off:1024],
                            mybir.ActivationFunctionType.Exp)
                    # PE filler while ACT evaluates exp
                    wave_no += 1
                    target = (fill_total * wave_no) // n_waves
                    while filler and fill_done < target:
                        filler.pop(0)()
                        fill_done += 1
                    if m >= 0:  # mask the 128-wide diagonal band
                        for half in (0, 512):
                            nc.vector.tensor_mul(
                                ew[:, half + off: half + off + 128],
                                ew[:, half + off: half + off + 128],
                                maskg_t[:, 640 * m: 640 * m + 128])
                    pending.append((ki, ew, off))
                    if len(pending) > 3:
                        emit_wv(*pending.pop(0))
                for item in pending:
                    emit_wv(*item)
                # normalization: 1/sums, broadcast via K=1 matmuls
                rec_t = sm_pool.tile([128, 1024], F32R, name="rec_t")
                nc.vector.reciprocal(rec_t[64:65, 0:512],
                                     h_ps[64:65, 0:512])
                nc.vector.reciprocal(rec_t[0:1, 512:1024],
                                     h_ps[0:1, 512:1024])
                bc_e = sh512.tile([128, 512], F32, name="bc_e", tag="sh512")
                nc.tensor.matmul(bc_e[0:64, :], ones_t[64:65, 0:64],
                                 rec_t[64:65, 0:512],
                                 start=True, stop=True,
                                 tile_position=(64, 0))
                bc_o = sh512.tile([128, 512], F32, name="bc_o", tag="sh512")
                nc.tensor.matmul(bc_o[:], ones_t[0:1, :],
                                 rec_t[0:1, 512:1024],
                                 start=True, stop=True,
                                 tile_position=(0, 0))
                bc_t = sm_pool.tile([128, 512], F32, name="bc_t")
                nc.vector.tensor_copy(bc_t[0:64, :], bc_e[0:64, :])
                nc.vector.tensor_copy(bc_t[64:128, :], bc_o[64:128, :])
                hn = hn_pool.tile([128, 512], F32R, name="hn")
                nc.vector.tensor_mul(hn[0:64, :], h_ps[0:64, 0:512],
                                     bc_t[0:64, :])
                nc.vector.tensor_mul(hn[64:128, :],
                                     h_ps[64:128, 512:1024],
                                     bc_t[64:128, :])
                hn_t[p] = hn
            # output projection of chunk J becomes filler for chunks J+1/J+2
            filler.extend(outproj_tasks(J, hn_t))
            if J + 1 < NQC:
                xt_cur = xt_next
        for t in filler:  # whatever is left (at least chunk 3's outproj)
            t()



class _Runner:
    """Jitted SPMD executor over the 8 axon-tunneled NeuronCores."""

    def __init__(self, nc, n_cores=N_CORES):
        import jax
        from jax.sharding import Mesh, PartitionSpec, NamedSharding
        from jax.experimental.shard_map import shard_map

        self.jax = jax
        bass2jax.install_neuronx_cc_hook()
        partition_name = (
            nc.partition_id_tensor.name if nc.partition_id_tensor else None
        )
        in_names, out_names, out_avals, zero_outs = [], [], [], []
        for alloc in nc.m.functions[0].allocations:
            if not isinstance(alloc, mybir.MemoryLocationSet):
                continue
            name = alloc.memorylocations[0].name
            if alloc.kind == "ExternalInput":
                if name != partition_name:
                    in_names.append(name)
            elif alloc.kind == "ExternalOutput":
                out_names.append(name)
                shape = tuple(alloc.tensor_shape)
                dtype = mybir.dt.np(alloc.dtype)
                out_avals.append(jax.core.ShapedArray(shape, dtype))
                zero_outs.append(np.zeros(shape, dtype))
        self.in_names = in_names
        self.out_names = out_names
        self.out_avals = out_avals
        self.zero_outs = zero_outs
        self.n_cores = n_cores
        all_in = list(in_names) + list(out_names)
        if partition_name is not None:
            all_in.append(partition_name)

        def _body(*args):
            operands = list(args)
            if partition_name is not None:
                operands.append(bass2jax.partition_id_tensor())
            outs = bass2jax._bass_exec_p.bind(
                *operands,
                out_avals=tuple(out_avals),
                in_names=tuple(all_in),
                out_names=tuple(out_names),
                lowering_input_output_aliases=(),
                sim_require_finite=True,
                sim_require_nnan=True,
                nc=nc,
            )
            return tuple(outs)

        devices = jax.devices()[:n_cores]
        assert len(devices) == n_cores
        self.mesh = Mesh(np.asarray(devices), ("core",))
        n_in = len(in_names) + len(out_names)
        self.fn = jax.jit(
            shard_map(
                _body, mesh=self.mesh,
                in_specs=(PartitionSpec("core"),) * n_in,
                out_specs=(PartitionSpec("core"),) * len(out_names),
                check_rep=False,
            ),
            keep_unused=True,
        )
        self.sharding = NamedSharding(self.mesh, PartitionSpec("core"))

    def put_inputs(self, in_maps):
        concat_in = [
            np.concatenate(
                [np.asarray(in_maps[c][n]) for c in range(self.n_cores)], axis=0
            )
            for n in self.in_names
        ]
        concat_zeros = [
            np.zeros((self.n_cores * z.shape[0], *z.shape[1:]), z.dtype)
            for z in self.zero_outs
        ]
        args = [
            self.jax.device_put(a, self.sharding)
            for a in concat_in + concat_zeros
        ]
        self.jax.block_until_ready(args)
        return args

    def run(self, args):
        out = self.fn(*args)
        self.jax.block_until_ready(out)
        return out

    def split_outputs(self, out_arrs):
        return [
            {
                n: np.asarray(out_arrs[i]).reshape(
                    self.n_cores, *self.out_avals[i].shape)[c]
                for i, n in enumerate(self.out_names)
            }
            for c in range(self.n_cores)
        ]


def make_core_inputs(x, Wq, bq, Wk, bk, Wv, bv, Wo):
    """Host-side slicing for the 8 cores. Wq/bq are pre-scaled by 1/sqrt(Dh)."""
    ones = np.ones((128, 128), np.float32)
    k_idx = np.arange(128)[:, None]
    q_idx = np.arange(512)[None, :]
    maskg = np.concatenate(
        [(k_idx <= q_idx - 128 * m).astype(np.float32) for m in range(4)],
        axis=1)
    in_maps = []
    xTb = [np.ascontiguousarray(x[b].T) for b in range(B)]
    for c in range(N_CORES):
        b, g = c // 4, c % 4
        fs = slice(g * FW, (g + 1) * FW)
        in_maps.append({
            "xT": xTb[b],
            "wq": np.ascontiguousarray(Wq[:, fs]),
            "wk": np.ascontiguousarray(Wk[:, fs]),
            "wv": np.ascontiguousarray(Wv[:, fs]),
            "wo": np.ascontiguousarray(Wo[fs, :]),
            "bq": np.ascontiguousarray(bq[fs]).reshape(FW, 1),
            "bk": np.ascontiguousarray(bk[fs]).reshape(FW, 1),
            "bvb": np.broadcast_to(bv[fs], (128, FW)).copy(),
            "ones": ones,
            "maskg": maskg,
        })
    return in_maps


_CACHE = {}


def get_runner(reps: int = 1, loop_trips: int = 1):
    key = (reps, loop_trips)
    if key not in _CACHE:
        _CACHE[key] = _Runner(build_nc(reps, loop_trips))
    return _CACHE[key]


def kernel(x, Wq, bq, Wk, bk, Wv, bv, Wo, bo):
    x = np.asarray(x, np.float32)
    scale = np.float32(1.0 / np.sqrt(DH))
    in_maps = make_core_inputs(
        x,
        np.asarray(Wq, np.float32) * scale, np.asarray(bq, np.float32) * scale,
        np.asarray(Wk, np.float32), np.asarray(bk, np.float32),
        np.asarray(Wv, np.float32), np.asarray(bv, np.float32),
        np.asarray(Wo, np.float32))
    r = get_runner()
    args = r.put_inputs(in_maps)
    outs = r.split_outputs(r.run(args))
    result = np.zeros((B, S, D), np.float32)
    for c in range(N_CORES):
        result[c // 4] += outs[c]["out"]
    result += np.asarray(bo, np.float32)
    return result



# revision 19
# speedup vs baseline: 1.3378x; 1.3378x over previous
"""Causal multi-head attention (B=2, S=2048, D=1024, H=16, Dh=64) on 8 trn2
NeuronCores.

Sharding: tensor-parallel over (batch x head-group). Core c handles batch
c//4 and heads [4*(c%4), 4*(c%4)+4). Each core computes its heads' Q/K/V
projections, causal softmax attention, and a partial output projection
(row-parallel Wo). Host sums the 4 partials per batch and adds bo.

Device-side layout ("scores-transposed"): the contraction dim always sits on
partitions so no transposes are ever needed:
  qT/kT: [head-dim on partitions, seq free]   (from W.T @ x.T)
  v:     [seq on partitions, head-dim free]   (from x @ Wv)
  scoresT[k, q] = kT-block.T @ qT-block       (k seq on partitions)
  softmax: exp on ACT (no max subtraction - scores are O(3) here); the row
           sums ride along the v matmul via an appended ones column; the raw
           sums are broadcast via K=1 matmuls (waiting only on a short scalar
           copy), inverted with one full-width single-pass DVE reciprocal,
           and applied with one vector mul. Reserved PE filler tasks cover
           the tail latency so the array never idles long enough for the
           HAM clock gate to re-throttle.
  out    = h_norm-blocks.T @ Wo-rows          (partial, summed on host)

All matmul operands are float32r (full PE rate at N>=256, ~1e-4 rel err).
Constraint honored throughout: matmul PSUM destinations must start at
partition 0, and a matmul with start=True zeroes its whole 2KB bank, so
even/odd head accumulation groups live in separate banks.
"""

import numpy as np

import concourse.bacc as bacc
import concourse.mybir as mybir
import concourse.tile as tile
from concourse import bass2jax

F32 = mybir.dt.float32
F32R = mybir.dt.float32r
BF16 = mybir.dt.bfloat16

B, S, D = 2, 2048, 1024
H_PER_CORE = 4          # heads per core
DH = 64                 # head dim
FW = H_PER_CORE * DH    # 256: per-core projection width
N_CORES = 8
QCHUNK = 512            # q columns processed per chunk
NQC = S // QCHUNK       # 4 chunks
KT = S // 128           # 16 k-tiles
# v_t per-s-tile layout, per head pair p at offset p*193:
#   [0:64]=v_even  [64:65]=1 (even sums row 64)  [65:66]=1 (odd sums row 0)
#   [66:129]=unused  [129:193]=v_odd (odd out rows 64:128)
VSEG = 193
VBLK = 2 * VSEG         # 386 per s-tile


def build_nc(reps: int = 1, loop_trips: int = 1):
    nc = bacc.Bacc("TRN2", target_bir_lowering=False, debug=False)

    xT = nc.dram_tensor("xT", [D, S], F32R, kind="ExternalInput")
    wq = nc.dram_tensor("wq", [D, FW], F32R, kind="ExternalInput")
    wk = nc.dram_tensor("wk", [D, FW], F32R, kind="ExternalInput")
    wv = nc.dram_tensor("wv", [D, FW], F32R, kind="ExternalInput")
    wo = nc.dram_tensor("wo", [FW, D], F32R, kind="ExternalInput")
    bq = nc.dram_tensor("bq", [FW, 1], F32, kind="ExternalInput")
    bk = nc.dram_tensor("bk", [FW, 1], F32, kind="ExternalInput")
    bvb = nc.dram_tensor("bvb", [128, FW], F32, kind="ExternalInput")
    ones = nc.dram_tensor("ones", [128, 128], F32R, kind="ExternalInput")
    maskg = nc.dram_tensor("maskg", [128, 2048], F32R, kind="ExternalInput")
    out = nc.dram_tensor("out", [S, D], F32, kind="ExternalOutput")

    with tile.TileContext(nc) as tc, nc.allow_low_precision(
            reason="float32r matmul operands carry reduced mantissas by design"):
        if loop_trips > 1:
            with tc.For_i(0, loop_trips, 1):
                _emit_body(nc, tc, xT, wq, wk, wv, wo, bq, bk, bvb, ones,
                           maskg, out)
        else:
            for _ in range(reps):
                _emit_body(nc, tc, xT, wq, wk, wv, wo, bq, bk, bvb, ones,
                           maskg, out)
    nc.compile()
    return nc


def _emit_body(nc, tc, xT, wq, wk, wv, wo, bq, bk, bvb, ones, maskg, out):
    """One full attention pass.

    Projections are emitted chunk-by-chunk and *interleaved into the previous
    chunk's softmax waves* (causality: attention chunk J only needs k/v/q
    chunks 0..J). Each engine executes its instructions in emission order, so
    the interleave is what keeps PE busy while ACT evaluates exp, and keeps
    the PE array HAM-warm. Output projection of chunk J is likewise deferred
    into chunk J+1's waves.
    """
    with tc.tile_pool(name="wpool", bufs=1) as wpool, \
         tc.tile_pool(name="qkv", bufs=1) as qkv, \
         tc.tile_pool(name="xtp", bufs=3) as xtp, \
         tc.tile_pool(name="spp", bufs=2, space="PSUM") as spp, \
         tc.tile_pool(name="hpp", bufs=1, space="PSUM") as hpp, \
         tc.tile_pool(name="sh512", bufs=2, space="PSUM") as sh512, \
         tc.tile_pool(name="expw", bufs=5) as expw_pool, \
         tc.tile_pool(name="sm", bufs=2) as sm_pool, \
         tc.tile_pool(name="hn", bufs=4) as hn_pool, \
         tc.tile_pool(name="op", bufs=4) as op_pool:
        # --- weight/aux tiles; DMAs ordered by first use ---
        wq_t = wpool.tile([128, 8 * FW], F32R)   # [d-in-tile, (d-tile, f)]
        wk_t = wpool.tile([128, 8 * FW], F32R)
        wv_t = wpool.tile([128, 8 * FW], F32R)
        wo_t = wpool.tile([128, 2 * D], F32R)    # [fw-in-tile, (fw-tile, n)]
        bq_t = wpool.tile([128, 2], F32)
        bk_t = wpool.tile([128, 2], F32)
        bvb_t = wpool.tile([128, FW], F32)
        ones_t = wpool.tile([128, 128], F32R)
        maskg_t = wpool.tile([128, 2048], F32R)
        qT = [qkv.tile([128, S], F32R, name=f"qT{p}") for p in range(2)]
        kT = [qkv.tile([128, S], F32R, name=f"kT{p}") for p in range(2)]
        v_t = qkv.tile([128, KT * VBLK], F32R)

        def dma_xt(J):
            x_t = xtp.tile([128, 8 * 512], F32R, name="xt")
            nc.sync.dma_start(
                out=x_t[:].rearrange("p (t s) -> p t s", t=8),
                in_=xT[:, J * 512:(J + 1) * 512].rearrange(
                    "(t p) s -> p t s", p=128))
            return [x_t[:, d * 512:(d + 1) * 512] for d in range(8)]

        # DMAs ordered by first use; one descriptor per tensor.
        nc.sync.dma_start(out=wk_t[:].rearrange("p (t f) -> p t f", t=8),
                          in_=wk[:].rearrange("(t p) f -> p t f", p=128))
        xt_cur = dma_xt(0)
        nc.sync.dma_start(out=wq_t[:].rearrange("p (t f) -> p t f", t=8),
                          in_=wq[:].rearrange("(t p) f -> p t f", p=128))
        nc.sync.dma_start(out=bk_t[:].rearrange("p (t f) -> p t f", t=2),
                          in_=bk[:].rearrange("(t p) f -> p t f", p=128))
        nc.sync.dma_start(out=bq_t[:].rearrange("p (t f) -> p t f", t=2),
                          in_=bq[:].rearrange("(t p) f -> p t f", p=128))
        nc.sync.dma_start(out=ones_t[:], in_=ones[:])
        nc.sync.dma_start(out=wv_t[:].rearrange("p (t f) -> p t f", t=8),
                          in_=wv[:].rearrange("(t p) f -> p t f", p=128))
        nc.sync.dma_start(out=bvb_t[:], in_=bvb[:])
        nc.sync.dma_start(out=maskg_t[:], in_=maskg[:])
        nc.sync.dma_start(out=wo_t[:].rearrange("p (t f) -> p t f", t=2),
                          in_=wo[:].rearrange("(t p) f -> p t f", p=128))
        # ones columns of v_t (positions 64,65 within each 193-block)
        nc.vector.tensor_copy(
            v_t[:].rearrange("x (s p b) -> x s p b", s=KT, p=2)[:, :, :, 64:66],
            ones_t[:, 0:64].rearrange("x (s p b) -> x s p b", s=KT, p=2))

        def proj_tasks(J, xt):
            """12 closures: one PE accumulation group + vector epilogue each."""
            tasks = []
            scs = slice(J * 512, (J + 1) * 512)
            for dst, w_t, b_t in ((kT, wk_t, bk_t), (qT, wq_t, bq_t)):
                for p in range(2):
                    def qk_group(dst=dst, w_t=w_t, b_t=b_t, p=p):
                        pt = sh512.tile([128, 512], F32, name="pt",
                                        tag="sh512")
                        for d in range(8):
                            nc.tensor.matmul(
                                pt[:],
                                w_t[:, d * FW + p * 128:
                                    d * FW + (p + 1) * 128],
                                xt[d][:],
                                start=(d == 0), stop=(d == 7),
                            )
                        nc.vector.tensor_scalar_add(
                            dst[p][:, scs], pt[:], b_t[:, p:p + 1])
                    tasks.append(qk_group)
            for j in range(4):
                def v_group(j=j):
                    st = 4 * J + j
                    pt = sh512.tile([128, FW], F32, name="pt", tag="sh512")
                    for d in range(8):
                        nc.tensor.matmul(
                            pt[:],
                            xt[d][:, j * 128:(j + 1) * 128],
                            wv_t[:, d * FW:(d + 1) * FW],
                            start=(d == 0), stop=(d == 7),
                        )
                    seg = v_t[:, st * VBLK:(st + 1) * VBLK].rearrange(
                        "x (p b) -> x p b", p=2)
                    pt4 = pt[:].rearrange("x (h c) -> x h c", h=4)
                    bv4 = bvb_t[:].rearrange("x (h c) -> x h c", h=4)
                    nc.vector.tensor_add(seg[:, :, 0:64], pt4[:, 0:4:2, :],
                                         bv4[:, 0:4:2, :])
                    nc.vector.tensor_add(seg[:, :, 129:193],
                                         pt4[:, 1:4:2, :], bv4[:, 1:4:2, :])
                tasks.append(v_group)
            return tasks

        def outproj_tasks(J, hn_t):
            """8 closures: 2-matmul group + copy; 4 DMA-out closures."""
            tasks = []
            o_tiles = {}
            for m in range(4):
                o_t = op_pool.tile([128, D], F32, name="o_t")
                o_tiles[m] = o_t
                for n in range(2):
                    def o_group(m=m, n=n, o_t=o_t):
                        o_ps = sh512.tile([128, 512], F32, name="o_ps",
                                          tag="sh512")
                        for p in range(2):
                            nc.tensor.matmul(
                                o_ps[:],
                                hn_t[p][:, m * 128:(m + 1) * 128],
                                wo_t[:, p * D + n * 512:
                                     p * D + (n + 1) * 512],
                                start=(p == 0), stop=(p == 1),
                            )
                        cp = nc.vector.tensor_copy if n == 0 else \
                            nc.scalar.copy
                        cp(o_t[:, n * 512:(n + 1) * 512], o_ps[:])
                        if n == 1:
                            nc.sync.dma_start(
                                out=out[J * 512 + m * 128:
                                        J * 512 + (m + 1) * 128, :],
                                in_=o_t[:])
                    tasks.append(o_group)
            return tasks

        filler = []
        for t in proj_tasks(0, xt_cur):
            t()  # chunk 0 projections run up front
        for J in range(NQC):
            n_ki = 4 * J + 4
            qs = slice(J * 512, (J + 1) * 512)
            if J + 1 < NQC:
                xt_next = dma_xt(J + 1)
                filler.extend(proj_tasks(J + 1, xt_next))
            n_waves = 2 * n_ki
            wave_no = 0
            fill_total = max(0, len(filler) - 4)  # reserve 4 for tails
            fill_done = 0
            hn_t = [None, None]
            for p in range(2):
                # h_ps bank 0: even head rows [0:64]=h, [64:65]=sums
                # h_ps bank 1: odd head  rows [0:1]=sums, [64:128]=h
                h_ps = hpp.tile([128, 1024], F32, name="h_ps")
                vbase = p * VSEG
                def emit_wv(ki, ew, off):
                    # h + sums in one matmul per head (ones col in v_t)
                    nc.tensor.matmul(
                        h_ps[0:65, off:512],
                        v_t[:, ki * VBLK + vbase: ki * VBLK + vbase + 65],
                        ew[:, off:512],
                        start=(ki == 0), stop=(ki == n_ki - 1),
                    )
                    nc.tensor.matmul(
                        h_ps[0:128, 512 + off:1024],
                        v_t[:, ki * VBLK + vbase + 65:
                            ki * VBLK + vbase + VSEG],
                        ew[:, 512 + off:1024],
                        start=(ki == 0), stop=(ki == n_ki - 1),
                    )

                # Software pipeline (lag 2): the wv matmuls of wave w are
                # emitted after the scores of wave w+2, so the PE stream
                # never waits on a freshly-issued exp (engines are FIFO).
                # Diagonal tiles (m >= 0) only touch columns [off:512],
                # off = 128*m: everything below is causally dead.
                pending = []
                for ki in range(n_ki):
                    m = ki - 4 * J
                    off = 128 * m if m > 0 else 0
                    sc_ps = spp.tile([128, 1024], F32, name="sc_ps")
                    # scores^T: row-tiled head pair (K=64 each)
                    nc.tensor.matmul(
                        sc_ps[:, off:512],
                        kT[p][0:64, ki * 128:(ki + 1) * 128],
                        qT[p][0:64, J * 512 + off:(J + 1) * 512],
                        start=True, stop=True, tile_position=(0, 0),
                    )
                    nc.tensor.matmul(
                        sc_ps[:, 512 + off:1024],
                        kT[p][64:128, ki * 128:(ki + 1) * 128],
                        qT[p][64:128, J * 512 + off:(J + 1) * 512],
                        start=True, stop=True, tile_position=(64, 0),
                    )
                    ew = expw_pool.tile([128, 1024], F32R, name="ew")
                    if off == 0:
                        nc.scalar.activation(
                            ew[:], sc_ps[:], mybir.ActivationFunctionType.Exp)
                    else:
                        nc.scalar.activation(
                            ew[:, off:512], sc_ps[:, off:512],
                            mybir.ActivationFunctionType.Exp)
                        nc.scalar.activation(
                            ew[:, 512 + off:1024], sc_ps[:, 512 + off:1024],
                            mybir.ActivationFunctionType.Exp)
                    # PE filler while ACT evaluates exp
                    wave_no += 1
                    target = (fill_total * wave_no) // n_waves
                    while filler and fill_done < target:
                        filler.pop(0)()
                        fill_done += 1
                    if m >= 0:  # mask the 128-wide diagonal band
                        for half in (0, 512):
                            nc.vector.tensor_mul(
                                ew[:, half + off: half + off + 128],
                                ew[:, half + off: half + off + 128],
                                maskg_t[:, 640 * m: 640 * m + 128])
                    pending.append((ki, ew, off))
                    if len(pending) > 3:
                        emit_wv(*pending.pop(0))
                for item in pending:
                    emit_wv(*item)
                # normalization: broadcast the RAW sums via K=1 matmuls
                # (they only wait on a short scalar copy, not a 3.4us DVE
                # reciprocal), then one full-width single-pass reciprocal on
                # the broadcast tile (~18 bits, far below tolerance; the
                # custom DVE op needs base partition 0 + many partitions).
                s_t = sm_pool.tile([128, 512], F32R, name="s_t")
                nc.scalar.activation(s_t[64:65, :], h_ps[64:65, 0:512],
                                     mybir.ActivationFunctionType.Copy)
                nc.scalar.activation(s_t[0:1, :], h_ps[0:1, 512:1024],
                                     mybir.ActivationFunctionType.Copy)
                # reserved PE filler so the array stays busy while the sum
                # staging runs (bc matmuls below wait on it)
                for _ in range(2):
                    if filler:
                        filler.pop(0)()
                        fill_done += 1
                bc_e = sh512.tile([128, 512], F32, name="bc_e", tag="sh512")
                nc.tensor.matmul(bc_e[0:64, :], ones_t[64:65, 0:64],
                                 s_t[64:65, :],
                                 start=True, stop=True,
                                 tile_position=(64, 0))
                bc_o = sh512.tile([128, 512], F32, name="bc_o", tag="sh512")
                nc.tensor.matmul(bc_o[:], ones_t[0:1, :],
                                 s_t[0:1, :],
                                 start=True, stop=True,
                                 tile_position=(0, 0))
                bc_t = sm_pool.tile([128, 512], F32, name="bc_t")
                nc.vector.tensor_copy(bc_t[0:64, :], bc_e[0:64, :])
                nc.scalar.copy(bc_t[64:128, :], bc_o[64:128, :])
                rec_t = sm_pool.tile([128, 512], F32, name="rec_t")
                nc.vector.reciprocal_approx_fast(rec_t[:], bc_t[:])
                hn = hn_pool.tile([128, 512], F32R, name="hn")
                nc.vector.tensor_mul(hn[0:64, :], h_ps[0:64, 0:512],
                                     rec_t[0:64, :])
                nc.vector.tensor_mul(hn[64:128, :],
                                     h_ps[64:128, 512:1024],
                                     rec_t[64:128, :])
                hn_t[p] = hn
            # output projection of chunk J becomes filler for chunks J+1/J+2
            filler.extend(outproj_tasks(J, hn_t))
            if J + 1 < NQC:
                xt_cur = xt_next
        for t in filler:  # whatever is left (at least chunk 3's outproj)
            t()



class _Runner:
    """Jitted SPMD executor over the 8 axon-tunneled NeuronCores."""

    def __init__(self, nc, n_cores=N_CORES):
        import jax
        from jax.sharding import Mesh, PartitionSpec, NamedSharding
        from jax.experimental.shard_map import shard_map

        self.jax = jax
        bass2jax.install_neuronx_cc_hook()
        partition_name = (
            nc.partition_id_tensor.name if nc.partition_id_tensor else None
        )
        in_names, out_names, out_avals, zero_outs = [], [], [], []
        for alloc in nc.m.functions[0].allocations:
            if not isinstance(alloc, mybir.MemoryLocationSet):
                continue
            name = alloc.memorylocations[0].name
            if alloc.kind == "ExternalInput":
                if name != partition_name:
                    in_names.append(name)
            elif alloc.kind == "ExternalOutput":
                out_names.append(name)
                shape = tuple(alloc.tensor_shape)
                dtype = mybir.dt.np(alloc.dtype)
                out_avals.append(jax.core.ShapedArray(shape, dtype))
                zero_outs.append(np.zeros(shape, dtype))
        self.in_names = in_names
        self.out_names = out_names
        self.out_avals = out_avals
        self.zero_outs = zero_outs
        self.n_cores = n_cores
        all_in = list(in_names) + list(out_names)
        if partition_name is not None:
            all_in.append(partition_name)

        def _body(*args):
            operands = list(args)
            if partition_name is not None:
                operands.append(bass2jax.partition_id_tensor())
            outs = bass2jax._bass_exec_p.bind(
                *operands,
                out_avals=tuple(out_avals),
                in_names=tuple(all_in),
                out_names=tuple(out_names),
                lowering_input_output_aliases=(),
                sim_require_finite=True,
                sim_require_nnan=True,
                nc=nc,
            )
            return tuple(outs)

        devices = jax.devices()[:n_cores]
        assert len(devices) == n_cores
        self.mesh = Mesh(np.asarray(devices), ("core",))
        n_in = len(in_names) + len(out_names)
        self.fn = jax.jit(
            shard_map(
                _body, mesh=self.mesh,
                in_specs=(PartitionSpec("core"),) * n_in,
                out_specs=(PartitionSpec("core"),) * len(out_names),
                check_rep=False,
            ),
            keep_unused=True,
        )
        self.sharding = NamedSharding(self.mesh, PartitionSpec("core"))

    def put_inputs(self, in_maps):
        concat_in = [
            np.concatenate(
                [np.asarray(in_maps[c][n]) for c in range(self.n_cores)], axis=0
            )
            for n in self.in_names
        ]
        concat_zeros = [
            np.zeros((self.n_cores * z.shape[0], *z.shape[1:]), z.dtype)
            for z in self.zero_outs
        ]
        args = [
            self.jax.device_put(a, self.sharding)
            for a in concat_in + concat_zeros
        ]
        self.jax.block_until_ready(args)
        return args

    def run(self, args):
        out = self.fn(*args)
        self.jax.block_until_ready(out)
        return out

    def split_outputs(self, out_arrs):
        return [
            {
                n: np.asarray(out_arrs[i]).reshape(
                    self.n_cores, *self.out_avals[i].shape)[c]
                for i, n in enumerate(self.out_names)
            }
            for c in range(self.n_cores)
        ]


def make_core_inputs(x, Wq, bq, Wk, bk, Wv, bv, Wo):
    """Host-side slicing for the 8 cores. Wq/bq are pre-scaled by 1/sqrt(Dh)."""
    ones = np.ones((128, 128), np.float32)
    k_idx = np.arange(128)[:, None]
    q_idx = np.arange(512)[None, :]
    maskg = np.concatenate(
        [(k_idx <= q_idx - 128 * m).astype(np.float32) for m in range(4)],
        axis=1)
    in_maps = []
    xTb = [np.ascontiguousarray(x[b].T) for b in range(B)]
    for c in range(N_CORES):
        b, g = c // 4, c % 4
        fs = slice(g * FW, (g + 1) * FW)
        in_maps.append({
            "xT": xTb[b],
            "wq": np.ascontiguousarray(Wq[:, fs]),
            "wk": np.ascontiguousarray(Wk[:, fs]),
            "wv": np.ascontiguousarray(Wv[:, fs]),
            "wo": np.ascontiguousarray(Wo[fs, :]),
            "bq": np.ascontiguousarray(bq[fs]).reshape(FW, 1),
            "bk": np.ascontiguousarray(bk[fs]).reshape(FW, 1),
            "bvb": np.broadcast_to(bv[fs], (128, FW)).copy(),
            "ones": ones,
            "maskg": maskg,
        })
    return in_maps


_CACHE = {}


def get_runner(reps: int = 1, loop_trips: int = 1):
    key = (reps, loop_trips)
    if key not in _CACHE:
        _CACHE[key] = _Runner(build_nc(reps, loop_trips))
    return _CACHE[key]


def kernel(x, Wq, bq, Wk, bk, Wv, bv, Wo, bo):
    x = np.asarray(x, np.float32)
    scale = np.float32(1.0 / np.sqrt(DH))
    in_maps = make_core_inputs(
        x,
        np.asarray(Wq, np.float32) * scale, np.asarray(bq, np.float32) * scale,
        np.asarray(Wk, np.float32), np.asarray(bk, np.float32),
        np.asarray(Wv, np.float32), np.asarray(bv, np.float32),
        np.asarray(Wo, np.float32))
    r = get_runner()
    args = r.put_inputs(in_maps)
    outs = r.split_outputs(r.run(args))
    result = np.zeros((B, S, D), np.float32)
    for c in range(N_CORES):
        result[c // 4] += outs[c]["out"]
    result += np.asarray(bo, np.float32)
    return result

